# revision 19
# baseline (speedup 1.0000x reference)
"""Trainium2 Bass kernel for nn_Amplified_PatternMixer.

Computation:
  out[b, h, m1, m2] = mixed_pattern[h, m1, m2] + alpha[h] * nrm[b, m2]
where
  nrm[b, m] = || mean_{hw}(x[b*57+m, :, h, w]) ||_2   over channels
  mixed_pattern = tiny 57x57 graph-normalized pattern (from 5x7x7 params).

The memory-bound part (streaming x: [1824, 256, 14, 14] f32, ~366 MB) runs
on 8 NeuronCores, data-parallel over rows (228 rows/core).

Per-core layout: each row (256 ch x 196 hw) is split into 4 quarter-rows of
64 channels; the 912 quarter-rows tile as 7 x [128 part, 64ch*196] plus one
[128 part, 8ch*196] tail (rows 224..227 split 32-ways).  Every tile's HBM
source is one fully contiguous range fanned over all 128 partitions, so all
16 SDMA engines carry identical byte loads (57 * 50176 B each).

Loads are issued via HWDGE (nc.sync/nc.scalar) rather than SWDGE
(nc.gpsimd): SWDGE descriptor generation runs on the GpSimd Q7 cores, which
are locked out of the shared SBUF port while DVE reduce ops run - the
baseline trace showed per-engine DMA rate collapsing from 27.1 GB/s to
~21 GB/s whenever vector was active.  HWDGE descriptors are generated in
RTL and are immune.

Each tile: vector reduce over hw=196 -> per-channel sums cs[128, w];
scalar Square-activation with accum_out -> per-piece sum of squares
(one partial per partition per tile).  Host combines the 4 (or 32)
partials per row, sqrt, /196.  The tiny 57x57 pattern-mixer runs on host.
"""

import numpy as np

import concourse.bacc as bacc
import concourse.mybir as mybir
import concourse.tile as tile
from concourse.bass_utils import run_bass_kernel_spmd

# Problem constants (hardcoded; kernel.py must be self-contained).
NUM_BASIC = 5
NUM_MIXED = 4
NUM_FRAME = 8
NUM_NODES = 7
NUM_SAMPLES = 8
M = 1 + NUM_NODES * NUM_FRAME  # 57

N_CORES = 8
B = 32
C = 256
HW = 196  # 14*14
ROWS_TOTAL = B * M          # 1824
ROWS_PER_CORE = ROWS_TOTAL // N_CORES  # 228
CW = C * HW                 # 50176 floats per row

# (row_start, w): tile covers rows row_start..row_start+w//2, as 128 pieces
# of w channels each (w/2 rows x 256/w pieces).  Per-partition run is
# w*196*4 bytes (<= 64KB keeps one descriptor per partition).
# Graded ramp + uniform 32ch body + tapered tail.  The ramp lets vector
# start ~1us after the first DMA lands; 32ch tiles hit the DVE reduce
# sweet spot (32-wide groups: 1.066 ns/elem vs 1.26 at 64-wide) and per
# tile vector work (6.7us) stays under DMA time (7.6us) so the pipeline
# is DMA-paced; deep buffering (bufs=4) keeps DMA issue from ever gating
# on vector; the taper keeps the post-DMA drain ~3us.  Best measured:
# 130.1us (quiet HBM); shared-HBM neighbor bursts add +10..25us to any
# config.  The 64ch-body + gpsimd-fold variant measured 133.0-133.4 in
# the same windows.
_WIDTHS = [4, 8, 16, 32] + [32] * 11 + [16, 16, 8, 4]
assert sum(_WIDTHS) == 2 * ROWS_PER_CORE
TILE_PLAN = []
_r = 0
for _w in _WIDTHS:
    TILE_PLAN.append((_r, _w))
    _r += _w // 2
N_TILES = len(TILE_PLAN)

# Per-tile GpSimd fold policy: number of channels whose hw-reduction is
# pre-halved (196 -> 98) by gpsimd.tensor_add before the vector reduce.
# Body 64-tiles fold half (raw reduce runs parallel to the fold); tail
# tiles fold (half for w=32, fully for w<=16) since gpsimd is idle there
# and vector is the drain path; the final tile stays raw (shortest
# latency chain).
def _fold_plan(widths):
    # With the 32ch body the vector engine keeps pace on raw reduces
    # (~95us total vs ~111us of DMA); gpsimd folding is not needed.
    return [0 for _ in widths]

_FOLDS = _fold_plan(_WIDTHS)

LAST_RESULT = None
_NC_CACHE = None


def _build_nc_hwdge(plan=TILE_PLAN, folds=_FOLDS, bufs=4,
                    rings=("sync", "scalar"), scalar_sq=True,
                    scalar_tiles=()):
    """HWDGE row-piece kernel.

    rings: cycle of engine names for the tile load DMAs (HWDGE engines:
    sync, scalar); alternating measured best.  Outstanding HWDGE DMAs
    time-share the SDMA engines (even on one queue), so completion sems
    fire ~(in-flight-behind bytes)/BW late -- another reason to keep
    tiles moderate and buffering deep.
    folds[t]: optional GpSimd pre-fold of the last folds[t] channels
    hw 196 -> 98 (tensor_add on its own SBUF door) to offload vector;
    unused in the default 32ch-body plan (vector keeps pace raw).
    bufs=4 ensures DMA issue (gated by consumer-done of the tile 4
    back) never waits on a vector hiccup.
    """
    nc = bacc.Bacc(None)
    x = nc.declare_dram_parameter(
        "x", [ROWS_PER_CORE, CW], mybir.dt.float32, isOutput=False
    )
    out = nc.declare_dram_parameter(
        "out", [128, len(plan)], mybir.dt.float32, isOutput=True
    )
    max_w = max(w for _, w in plan)
    max_fold = max(folds) if folds else 0
    scalar_tiles = scalar_tiles or ()
    fw = HW // 2  # 98
    with tile.TileContext(nc) as tc:
        with (
            tc.tile_pool(name="xt_pool", bufs=bufs) as xp,
            tc.tile_pool(name="fold_pool", bufs=2) as fp,
            tc.tile_pool(name="acc_pool", bufs=2) as accp,
            tc.tile_pool(name="res_pool", bufs=1) as resp,
            tc.tile_pool(name="trash_pool", bufs=2) as trp,
        ):
            osb = resp.tile([128, len(plan)], mybir.dt.float32, tag="osb")
            for t, (r0, w) in enumerate(plan):
                b = C // w          # pieces per row
                f = w * HW          # floats per partition
                xt = xp.tile([128, max_w * HW], mybir.dt.float32, tag="xt")
                if len(rings) == len(plan):
                    eng = getattr(nc, rings[t])
                else:
                    eng = getattr(nc, rings[t % len(rings)])
                src = x[r0 : r0 + w // 2, :].rearrange("a (b f) -> (a b) f", b=b)
                eng.dma_start(out=xt[:, :f], in_=src)
                cs = accp.tile([128, max_w], mybir.dt.float32, tag="cs")
                if t in scalar_tiles:
                    # Tail tiles: the scalar engine does the per-channel
                    # hw-sums (Copy-activation with accum_out), freeing the
                    # vector engine, which is backlogged at the drain.
                    tr = trp.tile([128, HW], mybir.dt.float32, tag="tr")
                    for ch in range(w):
                        nc.scalar.activation(
                            tr,
                            xt[:, ch * HW : (ch + 1) * HW],
                            mybir.ActivationFunctionType.Copy,
                            accum_out=cs[:, ch : ch + 1],
                        )
                    d = w
                else:
                    d = w - min(folds[t], w)
                    if d > 0:
                        nc.vector.reduce_sum(
                            cs[:, :d],
                            xt[:, : d * HW].rearrange("p (g w) -> p g w", w=HW),
                            axis=mybir.AxisListType.X,
                        )
                if d < w:
                    g = w - d
                    ft = fp.tile([128, max_fold * fw], mybir.dt.float32, tag="ft")
                    x3 = xt[:, d * HW : w * HW].rearrange(
                        "p (g w) -> p g w", w=HW
                    )
                    nc.gpsimd.tensor_add(
                        ft[:, : g * fw].rearrange("p (g w) -> p g w", w=fw),
                        x3[:, :, 0:fw],
                        x3[:, :, fw:HW],
                    )
                    nc.vector.reduce_sum(
                        cs[:, d:w],
                        ft[:, : g * fw].rearrange("p (g w) -> p g w", w=fw),
                        axis=mybir.AxisListType.X,
                    )
                if scalar_sq:
                    # Square + per-partition sum in one scalar op; keeps the
                    # vector engine free for the big reduces.
                    tr = accp.tile([128, max_w], mybir.dt.float32, tag="tr")
                    nc.scalar.activation(
                        tr[:, :w],
                        cs[:, :w],
                        mybir.ActivationFunctionType.Square,
                        accum_out=osb[:, t : t + 1],
                    )
                else:
                    sq = accp.tile([128, max_w], mybir.dt.float32, tag="sq")
                    nc.vector.tensor_mul(sq[:, :w], cs[:, :w], cs[:, :w])
                    nc.vector.reduce_sum(
                        osb[:, t : t + 1], sq[:, :w], axis=mybir.AxisListType.X
                    )
            nc.sync.dma_start(out=out[0:128, :], in_=osb)
    nc.finalize()
    return nc


def _get_nc():
    global _NC_CACHE
    if _NC_CACHE is None:
        _NC_CACHE = _build_nc_hwdge()
    return _NC_CACHE


def _norms_from_partials(partials):
    """partials: [128, N_TILES] per-core -> per-row sum of squares [228]."""
    nsq = np.zeros(ROWS_PER_CORE, dtype=np.float64)
    for t, (r0, w) in enumerate(TILE_PLAN):
        b = C // w
        ps = partials[:, t].astype(np.float64).reshape(w // 2, b).sum(axis=1)
        nsq[r0 : r0 + w // 2] += ps
    return np.sqrt(nsq) / float(HW)


def _zero_mask():
    mask = np.ones((M, M), dtype=np.float64)
    for i in range(NUM_SAMPLES):
        r = (1 + i) * NUM_NODES
        for c in range(1, M):
            if c % NUM_NODES != 0 and (c - 1) // NUM_NODES != i:
                mask[r, c] = 0.0
    return mask


def _pattern_mixer_np(mat, sigma, lin_w, lin_b, mixed_mat):
    mat = np.asarray(mat, np.float64)            # [5, 7, 7]
    sigma = np.asarray(sigma, np.float64)        # [4, 5, 1]
    lin_w = np.asarray(lin_w, np.float64)        # [4, 5]
    lin_b = np.asarray(lin_b, np.float64)        # [4]
    mixed_mat = np.asarray(mixed_mat, np.float64)  # [4, 57, 57]

    T2 = 2 * NUM_FRAME - 1  # 15
    dist = np.abs(np.arange(T2, dtype=np.float64) - (NUM_FRAME - 1))
    te = (1.0 / (np.sqrt(2.0 * np.pi) * sigma)) * np.exp(
        -(dist**2) / (2.0 * sigma**2)
    )  # [4, 5, 15]
    ce = 1.0 / (1.0 + np.exp(-te))
    mixed = (
        np.einsum("hbt,bnm,hb->hntm", ce, mat, lin_w)
        + lin_b[:, None, None, None]
    )
    mixed = np.maximum(mixed, 0.0).reshape(NUM_MIXED, NUM_NODES, T2 * NUM_NODES)
    blocks = [
        mixed[
            :,
            :,
            NUM_NODES * (NUM_SAMPLES - 1 - i) : NUM_NODES * (2 * NUM_SAMPLES - 1 - i),
        ]
        for i in range(NUM_SAMPLES)
    ]
    add_block = np.concatenate(blocks, axis=1)  # [4, 56, 56]
    mm = mixed_mat.copy()
    mm[:, 1:, 1:] += add_block
    mm *= _zero_mask()[None]
    deg = np.maximum(mm.sum(axis=2), 1.0) ** -0.5  # [4, 57]
    return (deg[:, :, None] * mm * deg[:, None, :]).astype(np.float32)


def kernel(mat, x, sigma, lin_w, lin_b, mixed_mat, alpha):
    global LAST_RESULT
    x = np.ascontiguousarray(np.asarray(x, dtype=np.float32))
    xs = x.reshape(ROWS_TOTAL, CW)
    in_maps = [
        {"x": xs[i * ROWS_PER_CORE : (i + 1) * ROWS_PER_CORE]} for i in range(N_CORES)
    ]
    nc = _get_nc()
    res = run_bass_kernel_spmd(nc, in_maps, core_ids=list(range(N_CORES)))
    LAST_RESULT = res
    norms = np.concatenate([_norms_from_partials(r["out"]) for r in res.results])
    nrm = norms.reshape(B, M).astype(np.float32)

    mp = _pattern_mixer_np(mat, sigma, lin_w, lin_b, mixed_mat)  # [4, 57, 57] f32
    alpha = np.asarray(alpha, np.float32).reshape(1, NUM_MIXED, 1, 1)
    out = mp[None] + alpha * nrm[:, None, None, :]
    return np.ascontiguousarray(out.astype(np.float32))
